# revision 21
# baseline (speedup 1.0000x reference)
"""Grouped SwiGLU expert FFN (MoE) on 8 Trainium2 NeuronCores.

Expert parallelism: expert e's weights + its (pre-sorted) token slice go to
core e. Each core runs x@w1, x@w3, silu/mul, h@w2 for its 8192 tokens.

v5: fp16 datapath with host-side input cast; PE does pure GEMM.
 - kernel() casts x and w1/w2/w3 to fp16 in numpy before upload, so the
   device sees fp16 ExternalInputs (half the HBM traffic, no on-chip casts).
 - x is loaded TRANSPOSED straight from HBM by the DMA xbar transpose
   (HWDGE, SP queue; 2-byte dtype required - hence the host cast).  A 3D
   out AP [128, 8, 512] enumerates transposed rows c-major (i = c*128+p),
   matching the "(c p)" weight layout (verified on HW by probe_xbar.py).
 - the PE never transposes: 3072 matmuls x 512-wide moving operands at
   1 cycle/row (fp16) is the whole tensor-engine program.

Math per core (dims: t=tokens, i=dim_in, j=dim_hid, o=dim_in):
  mm1/mm3: psum[j,t] += lhsT=w{1,3}[i_chunk, j_chunk] (stationary),
           rhs=xT[i_chunk, t_block] (moving 512) -> h1T/h3T.
  SwiGLU:  hT = silu(h1T) * h3T  (ACT Silu -> fp16, DVE mul -> fp16).
  mm2:     lhsT=hT[j_chunk, t_chunk] (stationary), rhs=w2[j_chunk, o_block]
           (moving 512) -> psum[t,o] natural-layout f32 output.
"""

import sys

sys.path.insert(0, "/opt/trn_rl_repo")

import numpy as np

N_CORES = 8
D = 1024  # dim_in
H = 1024  # dim_hid
P = 128
TB = 512  # token block per pipeline stage

_CACHE = {}


def _build(tok):
    import concourse.bacc as bacc
    import concourse.tile as tile
    from concourse import mybir

    dt = mybir.dt
    AF = mybir.ActivationFunctionType
    f32 = dt.float32
    f16 = dt.float16

    assert tok % TB == 0
    n_blk = tok // TB
    n_i = D // P   # 8 contraction chunks for mm1/mm3
    n_j = H // P   # 8 contraction chunks for mm2
    n_tc = TB // P  # 4 token chunks per block
    n_o = D // 512  # 2 output column blocks

    nc = bacc.Bacc(trn_type="TRN2", target_bir_lowering=False)
    x_h = nc.dram_tensor("x16", [tok, D], f16, kind="ExternalInput")
    w1_h = nc.dram_tensor("w1", [D, H], f16, kind="ExternalInput")
    w2_h = nc.dram_tensor("w2", [H, D], f16, kind="ExternalInput")
    w3_h = nc.dram_tensor("w3", [D, H], f16, kind="ExternalInput")
    out_h = nc.dram_tensor("out", [tok, D], f32, kind="ExternalOutput")

    with tile.TileContext(nc) as tc:
        with (
            tc.tile_pool(name="wpool", bufs=1) as wpool,
            tc.tile_pool(name="xtpool", bufs=4) as xtpool,
            tc.tile_pool(name="htpool", bufs=2) as htpool,
            tc.tile_pool(name="spool", bufs=3) as spool,
            tc.tile_pool(name="opool", bufs=2) as opool,
            tc.tile_pool(name="pAB", bufs=4, space="PSUM") as pABp,
            tc.tile_pool(name="pC", bufs=4, space="PSUM") as pCp,
        ):
            # Resident fp16 weights, 2KB-contiguous rows.  Startup transfers
            # are spread across all three DMA-capable queues so block 0's
            # dependencies (xT0, w1 cols 0:512, then w3, then w2) all land
            # before the matmuls need them:
            #  - gpsimd (else empty): w1 in column halves, then w2
            #  - sync: w3 ahead of transposes T1.. (one xbar-mode transition,
            #    which only delays T1 - not needed until ~60us)
            #  - scalar: T0 (ACT's preamble ends ~4us before SP's, so xT0 is
            #    ready ~13us instead of ~21us)
            w1s = wpool.tile([P, n_i, H], f16)
            w3s = wpool.tile([P, n_i, H], f16)
            w2s = wpool.tile([P, n_j, D], f16)

            o_r = out_h[:, :].rearrange("(b c p) d -> b p c d", p=P, c=n_tc)

            for b in range(n_blk):
                # ---- xbar-transpose load: xT[p, c, t] = x[b*TB+t, c*128+p]
                xT = xtpool.tile([P, n_i, TB], f16)
                eng = nc.scalar if b == 0 else nc.sync
                eng.dma_start(
                    out=xT, in_=x_h[b * TB:(b + 1) * TB, :], transpose=True
                )
                if b == 0:
                    # xbar-mode transitions serialize copy<->transpose DMAs
                    # GLOBALLY (measured: T0 waits all in-flight copies; the
                    # next copies wait T0).  Cross-queue emission order does
                    # NOT control dispatch order, so: gpsimd carries ONLY
                    # w1's first half (the one copy T0 must wait out), and
                    # every other weight copy rides the scalar queue BEHIND
                    # T0 in program order, first-needed first.
                    nc.gpsimd.dma_start(
                        out=w1s[:, :, 0:512],
                        in_=w1_h[:, 0:512].rearrange("(c p) h -> p c h", p=P),
                    )
                    nc.scalar.dma_start(
                        out=w3s[:, :, 0:512],
                        in_=w3_h[:, 0:512].rearrange("(c p) h -> p c h", p=P),
                    )
                    nc.scalar.dma_start(
                        out=w1s[:, :, 512:1024],
                        in_=w1_h[:, 512:1024].rearrange(
                            "(c p) h -> p c h", p=P
                        ),
                    )
                    nc.scalar.dma_start(
                        out=w3s[:, :, 512:1024],
                        in_=w3_h[:, 512:1024].rearrange(
                            "(c p) h -> p c h", p=P
                        ),
                    )
                    nc.scalar.dma_start(
                        out=w2s,
                        in_=w2_h[:, :].rearrange("(c p) h -> p c h", p=P),
                    )

                # ---- mm1/mm3 + SwiGLU -> hT [P(=j in chunk), n_j, TB] fp16
                hT = htpool.tile([P, n_j, TB], f16)
                for j in range(n_j):
                    pA = pABp.tile([P, TB], f32, tag="pAB")
                    pB = pABp.tile([P, TB], f32, tag="pAB")
                    for i in range(n_i):
                        nc.tensor.matmul(
                            pA, w1s[:, i, j * P:(j + 1) * P], xT[:, i, :],
                            start=(i == 0), stop=(i == n_i - 1),
                        )
                    for i in range(n_i):
                        nc.tensor.matmul(
                            pB, w3s[:, i, j * P:(j + 1) * P], xT[:, i, :],
                            start=(i == 0), stop=(i == n_i - 1),
                        )
                    s1 = spool.tile([P, TB], f16)
                    nc.scalar.activation(s1, pA, AF.Silu)
                    nc.vector.tensor_mul(hT[:, j, :], pB, s1)

                # ---- mm2 -> natural-layout out block; DMA per t-chunk so
                # the last block's store overlaps its own mm2.
                o_sb = opool.tile([P, n_tc, D], f32)
                for t in range(n_tc):
                    for o in range(n_o):
                        pC = pCp.tile([P, 512], f32)
                        for j in range(n_j):
                            nc.tensor.matmul(
                                pC,
                                hT[:, j, t * P:(t + 1) * P],
                                w2s[:, j, o * 512:(o + 1) * 512],
                                start=(j == 0), stop=(j == n_j - 1),
                            )
                        nc.scalar.activation(
                            o_sb[:, t, o * 512:(o + 1) * 512], pC, AF.Copy
                        )
                    nc.scalar.dma_start(
                        out=o_r[b, :, t, :], in_=o_sb[:, t, :]
                    )

    nc.compile()
    return nc


def _get_nc(tok):
    if tok not in _CACHE:
        _CACHE[tok] = _build(tok)
    return _CACHE[tok]


def _prep(x, w1, w2, w3, m_sizes):
    """Shared host-side prep: fp16 casts, per-expert sharding, padding.

    Returns (nc, in_maps, sizes)."""
    x = np.asarray(x)
    sizes = np.asarray(m_sizes).astype(np.int64)
    offs = np.concatenate([[0], np.cumsum(sizes)])
    n_exp = sizes.shape[0]
    assert n_exp == N_CORES

    pad = int(max(int(sizes.max()), TB))
    pad = ((pad + TB - 1) // TB) * TB
    nc = _get_nc(pad)

    x16 = x.astype(np.float16)
    w116 = np.asarray(w1).astype(np.float16)
    w216 = np.asarray(w2).astype(np.float16)
    w316 = np.asarray(w3).astype(np.float16)

    in_maps = []
    for e in range(N_CORES):
        xe = x16[offs[e]:offs[e + 1]]
        if xe.shape[0] < pad:
            xe = np.concatenate(
                [xe, np.zeros((pad - xe.shape[0], D), dtype=np.float16)], axis=0
            )
        in_maps.append({"x16": xe, "w1": w116[e], "w2": w216[e], "w3": w316[e]})
    return nc, in_maps, sizes


def kernel(x, w1, w2, w3, m_sizes):
    from concourse.bass_utils import run_bass_kernel_spmd

    nc, in_maps, sizes = _prep(x, w1, w2, w3, m_sizes)
    r = run_bass_kernel_spmd(nc, in_maps, core_ids=list(range(N_CORES)))
    out = np.concatenate(
        [r.results[e]["out"][: sizes[e]] for e in range(N_CORES)], axis=0
    )
    return out.astype(np.float32)


# revision 24
# speedup vs baseline: 1.0081x; 1.0081x over previous
"""Grouped SwiGLU expert FFN (MoE) on 8 Trainium2 NeuronCores.

Expert parallelism: expert e's weights + its (pre-sorted) token slice go to
core e. Each core runs x@w1, x@w3, silu/mul, h@w2 for its 8192 tokens.

v5: fp16 datapath with host-side input cast; PE does pure GEMM.
 - kernel() casts x and w1/w2/w3 to fp16 in numpy before upload, so the
   device sees fp16 ExternalInputs (half the HBM traffic, no on-chip casts).
 - x is loaded TRANSPOSED straight from HBM by the DMA xbar transpose
   (HWDGE, SP queue; 2-byte dtype required - hence the host cast).  A 3D
   out AP [128, 8, 512] enumerates transposed rows c-major (i = c*128+p),
   matching the "(c p)" weight layout (verified on HW by probe_xbar.py).
 - the PE never transposes: 3072 matmuls x 512-wide moving operands at
   1 cycle/row (fp16) is the whole tensor-engine program.

Math per core (dims: t=tokens, i=dim_in, j=dim_hid, o=dim_in):
  mm1/mm3: psum[j,t] += lhsT=w{1,3}[i_chunk, j_chunk] (stationary),
           rhs=xT[i_chunk, t_block] (moving 512) -> h1T/h3T.
  SwiGLU:  hT = silu(h1T) * h3T  (ACT Silu -> fp16, DVE mul -> fp16).
  mm2:     lhsT=hT[j_chunk, t_chunk] (stationary), rhs=w2[j_chunk, o_block]
           (moving 512) -> psum[t,o] natural-layout f32 output.
"""

import sys

sys.path.insert(0, "/opt/trn_rl_repo")

import numpy as np

N_CORES = 8
D = 1024  # dim_in
H = 1024  # dim_hid
P = 128
TB = 512  # token block per pipeline stage

_CACHE = {}


def _build(tok):
    import concourse.bacc as bacc
    import concourse.tile as tile
    from concourse import mybir

    dt = mybir.dt
    AF = mybir.ActivationFunctionType
    f32 = dt.float32
    f16 = dt.float16

    assert tok % TB == 0
    n_blk = tok // TB
    n_i = D // P   # 8 contraction chunks for mm1/mm3
    n_j = H // P   # 8 contraction chunks for mm2
    n_tc = TB // P  # 4 token chunks per block
    n_o = D // 512  # 2 output column blocks

    nc = bacc.Bacc(trn_type="TRN2", target_bir_lowering=False)
    x_h = nc.dram_tensor("x16", [tok, D], f16, kind="ExternalInput")
    w1_h = nc.dram_tensor("w1", [D, H], f16, kind="ExternalInput")
    w2_h = nc.dram_tensor("w2", [H, D], f16, kind="ExternalInput")
    w3_h = nc.dram_tensor("w3", [D, H], f16, kind="ExternalInput")
    out_h = nc.dram_tensor("out", [tok, D], f32, kind="ExternalOutput")

    with tile.TileContext(nc) as tc:
        with (
            tc.tile_pool(name="wpool", bufs=1) as wpool,
            tc.tile_pool(name="xtpool", bufs=4) as xtpool,
            tc.tile_pool(name="htpool", bufs=2) as htpool,
            tc.tile_pool(name="spool", bufs=3) as spool,
            tc.tile_pool(name="opool", bufs=2) as opool,
            tc.tile_pool(name="pAB", bufs=4, space="PSUM") as pABp,
            tc.tile_pool(name="pC", bufs=4, space="PSUM") as pCp,
        ):
            # Resident fp16 weights, 2KB-contiguous rows.  Startup transfers
            # are spread across all three DMA-capable queues so block 0's
            # dependencies (xT0, w1 cols 0:512, then w3, then w2) all land
            # before the matmuls need them:
            #  - gpsimd (else empty): w1 in column halves, then w2
            #  - sync: w3 ahead of transposes T1.. (one xbar-mode transition,
            #    which only delays T1 - not needed until ~60us)
            #  - scalar: T0 (ACT's preamble ends ~4us before SP's, so xT0 is
            #    ready ~13us instead of ~21us)
            w1s = wpool.tile([P, n_i, H], f16)
            w3s = wpool.tile([P, n_i, H], f16)
            w2s = wpool.tile([P, n_j, D], f16)
            # Wave 1 (concurrent, ~3MiB): w1 cols 0:512 (gpsimd) + w3 cols
            # 0:512 (sync) + T0 (scalar, below) — everything mm1/mm3 j=0-3
            # needs.  Wave 2: the second halves and w2 (scalar, behind T0).
            # Pre-T0 copy group all on gpsimd: Tile's global xbar ordering
            # put gpsimd copies BEFORE T0 but sync-queue copies AFTER it
            # (copy<->transpose groups serialize globally), which left mm3
            # stalled 8us on w3.  w1a+w3a+w1b ride gpsimd; w3b+w2 follow T0
            # on the scalar queue.
            nc.gpsimd.dma_start(
                out=w1s[:, :, 0:512],
                in_=w1_h[:, 0:512].rearrange("(c p) h -> p c h", p=P),
            )
            nc.gpsimd.dma_start(
                out=w3s[:, :, 0:512],
                in_=w3_h[:, 0:512].rearrange("(c p) h -> p c h", p=P),
            )
            nc.gpsimd.dma_start(
                out=w1s[:, :, 512:1024],
                in_=w1_h[:, 512:1024].rearrange("(c p) h -> p c h", p=P),
            )

            o_r = out_h[:, :].rearrange("(b c p) d -> b p c d", p=P, c=n_tc)

            for b in range(n_blk):
                # ---- xbar-transpose load: xT[p, c, t] = x[b*TB+t, c*128+p]
                xT = xtpool.tile([P, n_i, TB], f16)
                eng = nc.scalar if b == 0 else nc.sync
                eng.dma_start(
                    out=xT, in_=x_h[b * TB:(b + 1) * TB, :], transpose=True
                )
                if b == 0:
                    # Post-T0 copies on the scalar queue, first-needed first:
                    # w3 cols 512:1024 (mm3 j=4+, ~28us) then w2 (~45us).
                    nc.scalar.dma_start(
                        out=w3s[:, :, 512:1024],
                        in_=w3_h[:, 512:1024].rearrange(
                            "(c p) h -> p c h", p=P
                        ),
                    )
                    nc.scalar.dma_start(
                        out=w2s,
                        in_=w2_h[:, :].rearrange("(c p) h -> p c h", p=P),
                    )

                # ---- mm1/mm3 + SwiGLU -> hT [P(=j in chunk), n_j, TB] fp16
                hT = htpool.tile([P, n_j, TB], f16)
                for j in range(n_j):
                    pA = pABp.tile([P, TB], f32, tag="pAB")
                    pB = pABp.tile([P, TB], f32, tag="pAB")
                    for i in range(n_i):
                        nc.tensor.matmul(
                            pA, w1s[:, i, j * P:(j + 1) * P], xT[:, i, :],
                            start=(i == 0), stop=(i == n_i - 1),
                        )
                    for i in range(n_i):
                        nc.tensor.matmul(
                            pB, w3s[:, i, j * P:(j + 1) * P], xT[:, i, :],
                            start=(i == 0), stop=(i == n_i - 1),
                        )
                    s1 = spool.tile([P, TB], f16)
                    nc.scalar.activation(s1, pA, AF.Silu)
                    nc.vector.tensor_mul(hT[:, j, :], pB, s1)

                # ---- mm2 -> natural-layout out block; DMA per t-chunk so
                # the last block's store overlaps its own mm2.
                o_sb = opool.tile([P, n_tc, D], f32)
                for t in range(n_tc):
                    for o in range(n_o):
                        pC = pCp.tile([P, 512], f32)
                        for j in range(n_j):
                            nc.tensor.matmul(
                                pC,
                                hT[:, j, t * P:(t + 1) * P],
                                w2s[:, j, o * 512:(o + 1) * 512],
                                start=(j == 0), stop=(j == n_j - 1),
                            )
                        nc.scalar.activation(
                            o_sb[:, t, o * 512:(o + 1) * 512], pC, AF.Copy
                        )
                    nc.scalar.dma_start(
                        out=o_r[b, :, t, :], in_=o_sb[:, t, :]
                    )

    nc.compile()
    return nc


def _get_nc(tok):
    if tok not in _CACHE:
        _CACHE[tok] = _build(tok)
    return _CACHE[tok]


def _prep(x, w1, w2, w3, m_sizes):
    """Shared host-side prep: fp16 casts, per-expert sharding, padding.

    Returns (nc, in_maps, sizes)."""
    x = np.asarray(x)
    sizes = np.asarray(m_sizes).astype(np.int64)
    offs = np.concatenate([[0], np.cumsum(sizes)])
    n_exp = sizes.shape[0]
    assert n_exp == N_CORES

    pad = int(max(int(sizes.max()), TB))
    pad = ((pad + TB - 1) // TB) * TB
    nc = _get_nc(pad)

    x16 = x.astype(np.float16)
    w116 = np.asarray(w1).astype(np.float16)
    w216 = np.asarray(w2).astype(np.float16)
    w316 = np.asarray(w3).astype(np.float16)

    in_maps = []
    for e in range(N_CORES):
        xe = x16[offs[e]:offs[e + 1]]
        if xe.shape[0] < pad:
            xe = np.concatenate(
                [xe, np.zeros((pad - xe.shape[0], D), dtype=np.float16)], axis=0
            )
        in_maps.append({"x16": xe, "w1": w116[e], "w2": w216[e], "w3": w316[e]})
    return nc, in_maps, sizes


def kernel(x, w1, w2, w3, m_sizes):
    from concourse.bass_utils import run_bass_kernel_spmd

    nc, in_maps, sizes = _prep(x, w1, w2, w3, m_sizes)
    r = run_bass_kernel_spmd(nc, in_maps, core_ids=list(range(N_CORES)))
    out = np.concatenate(
        [r.results[e]["out"][: sizes[e]] for e in range(N_CORES)], axis=0
    )
    return out.astype(np.float32)


# revision 26
# speedup vs baseline: 1.0190x; 1.0108x over previous
"""Grouped SwiGLU expert FFN (MoE) on 8 Trainium2 NeuronCores.

Expert parallelism: expert e's weights + its (pre-sorted) token slice go to
core e. Each core runs x@w1, x@w3, silu/mul, h@w2 for its 8192 tokens.

v5: fp16 datapath with host-side input cast; PE does pure GEMM.
 - kernel() casts x and w1/w2/w3 to fp16 in numpy before upload, so the
   device sees fp16 ExternalInputs (half the HBM traffic, no on-chip casts).
 - x is loaded TRANSPOSED straight from HBM by the DMA xbar transpose
   (HWDGE, SP queue; 2-byte dtype required - hence the host cast).  A 3D
   out AP [128, 8, 512] enumerates transposed rows c-major (i = c*128+p),
   matching the "(c p)" weight layout (verified on HW by probe_xbar.py).
 - the PE never transposes: 3072 matmuls x 512-wide moving operands at
   1 cycle/row (fp16) is the whole tensor-engine program.

Math per core (dims: t=tokens, i=dim_in, j=dim_hid, o=dim_in):
  mm1/mm3: psum[j,t] += lhsT=w{1,3}[i_chunk, j_chunk] (stationary),
           rhs=xT[i_chunk, t_block] (moving 512) -> h1T/h3T.
  SwiGLU:  hT = silu(h1T) * h3T  (ACT Silu -> fp16, DVE mul -> fp16).
  mm2:     lhsT=hT[j_chunk, t_chunk] (stationary), rhs=w2[j_chunk, o_block]
           (moving 512) -> psum[t,o] natural-layout f32 output.
"""

import sys

sys.path.insert(0, "/opt/trn_rl_repo")

import numpy as np

N_CORES = 8
D = 1024  # dim_in
H = 1024  # dim_hid
P = 128
TB = 512  # token block per pipeline stage

_CACHE = {}


def _build(tok):
    import concourse.bacc as bacc
    import concourse.tile as tile
    from concourse import mybir

    dt = mybir.dt
    AF = mybir.ActivationFunctionType
    f32 = dt.float32
    f16 = dt.float16

    assert tok % TB == 0
    n_blk = tok // TB
    n_i = D // P   # 8 contraction chunks for mm1/mm3
    n_j = H // P   # 8 contraction chunks for mm2
    n_tc = TB // P  # 4 token chunks per block
    n_o = D // 512  # 2 output column blocks

    nc = bacc.Bacc(trn_type="TRN2", target_bir_lowering=False)
    x_h = nc.dram_tensor("x16", [tok, D], f16, kind="ExternalInput")
    w1_h = nc.dram_tensor("w1", [D, H], f16, kind="ExternalInput")
    w2_h = nc.dram_tensor("w2", [H, D], f16, kind="ExternalInput")
    w3_h = nc.dram_tensor("w3", [D, H], f16, kind="ExternalInput")
    out_h = nc.dram_tensor("out", [tok, D], f32, kind="ExternalOutput")

    with tile.TileContext(nc) as tc:
        with (
            tc.tile_pool(name="wpool", bufs=1) as wpool,
            tc.tile_pool(name="xtpool", bufs=5) as xtpool,
            tc.tile_pool(name="htpool", bufs=3) as htpool,
            tc.tile_pool(name="spool", bufs=3) as spool,
            tc.tile_pool(name="opool", bufs=3) as opool,
            tc.tile_pool(name="pAB", bufs=4, space="PSUM") as pABp,
            tc.tile_pool(name="pC", bufs=4, space="PSUM") as pCp,
        ):
            # Resident fp16 weights, 2KB-contiguous rows.  Startup transfers
            # are spread across all three DMA-capable queues so block 0's
            # dependencies (xT0, w1 cols 0:512, then w3, then w2) all land
            # before the matmuls need them:
            #  - gpsimd (else empty): w1 in column halves, then w2
            #  - sync: w3 ahead of transposes T1.. (one xbar-mode transition,
            #    which only delays T1 - not needed until ~60us)
            #  - scalar: T0 (ACT's preamble ends ~4us before SP's, so xT0 is
            #    ready ~13us instead of ~21us)
            w1s = wpool.tile([P, n_i, H], f16)
            w3s = wpool.tile([P, n_i, H], f16)
            w2s = wpool.tile([P, n_j, D], f16)
            # Wave 1 (concurrent, ~3MiB): w1 cols 0:512 (gpsimd) + w3 cols
            # 0:512 (sync) + T0 (scalar, below) — everything mm1/mm3 j=0-3
            # needs.  Wave 2: the second halves and w2 (scalar, behind T0).
            nc.gpsimd.dma_start(
                out=w1s[:, :, 0:512],
                in_=w1_h[:, 0:512].rearrange("(c p) h -> p c h", p=P),
            )
            nc.sync.dma_start(
                out=w3s[:, :, 0:512],
                in_=w3_h[:, 0:512].rearrange("(c p) h -> p c h", p=P),
            )
            nc.gpsimd.dma_start(
                out=w1s[:, :, 512:1024],
                in_=w1_h[:, 512:1024].rearrange("(c p) h -> p c h", p=P),
            )
            nc.sync.dma_start(
                out=w3s[:, :, 512:1024],
                in_=w3_h[:, 512:1024].rearrange("(c p) h -> p c h", p=P),
            )

            o_r = out_h[:, :].rearrange("(b c p) d -> b p c d", p=P, c=n_tc)

            for b in range(n_blk):
                # ---- xbar-transpose load: xT[p, c, t] = x[b*TB+t, c*128+p]
                xT = xtpool.tile([P, n_i, TB], f16)
                eng = nc.scalar if b == 0 else nc.sync
                eng.dma_start(
                    out=xT, in_=x_h[b * TB:(b + 1) * TB, :], transpose=True
                )
                if b == 0:
                    # w2 rides the scalar queue behind T0: not needed until
                    # mm2(0) (~40us), and launching it at t=0 starves T0.
                    nc.scalar.dma_start(
                        out=w2s,
                        in_=w2_h[:, :].rearrange("(c p) h -> p c h", p=P),
                    )

                # ---- mm1/mm3 + SwiGLU -> hT [P(=j in chunk), n_j, TB] fp16
                hT = htpool.tile([P, n_j, TB], f16)
                for j in range(n_j):
                    pA = pABp.tile([P, TB], f32, tag="pAB")
                    pB = pABp.tile([P, TB], f32, tag="pAB")
                    for i in range(n_i):
                        nc.tensor.matmul(
                            pA, w1s[:, i, j * P:(j + 1) * P], xT[:, i, :],
                            start=(i == 0), stop=(i == n_i - 1),
                        )
                    for i in range(n_i):
                        nc.tensor.matmul(
                            pB, w3s[:, i, j * P:(j + 1) * P], xT[:, i, :],
                            start=(i == 0), stop=(i == n_i - 1),
                        )
                    s1 = spool.tile([P, TB], f16)
                    nc.scalar.activation(s1, pA, AF.Silu)
                    nc.vector.tensor_mul(hT[:, j, :], pB, s1)

                # ---- mm2 -> natural-layout out block; DMA per t-chunk so
                # the last block's store overlaps its own mm2.
                o_sb = opool.tile([P, n_tc, D], f32)
                for t in range(n_tc):
                    for o in range(n_o):
                        pC = pCp.tile([P, 512], f32)
                        for j in range(n_j):
                            nc.tensor.matmul(
                                pC,
                                hT[:, j, t * P:(t + 1) * P],
                                w2s[:, j, o * 512:(o + 1) * 512],
                                start=(j == 0), stop=(j == n_j - 1),
                            )
                        nc.scalar.activation(
                            o_sb[:, t, o * 512:(o + 1) * 512], pC, AF.Copy
                        )
                    nc.scalar.dma_start(
                        out=o_r[b, :, t, :], in_=o_sb[:, t, :]
                    )

    nc.compile()
    return nc


def _get_nc(tok):
    if tok not in _CACHE:
        _CACHE[tok] = _build(tok)
    return _CACHE[tok]


def _prep(x, w1, w2, w3, m_sizes):
    """Shared host-side prep: fp16 casts, per-expert sharding, padding.

    Returns (nc, in_maps, sizes)."""
    x = np.asarray(x)
    sizes = np.asarray(m_sizes).astype(np.int64)
    offs = np.concatenate([[0], np.cumsum(sizes)])
    n_exp = sizes.shape[0]
    assert n_exp == N_CORES

    pad = int(max(int(sizes.max()), TB))
    pad = ((pad + TB - 1) // TB) * TB
    nc = _get_nc(pad)

    x16 = x.astype(np.float16)
    w116 = np.asarray(w1).astype(np.float16)
    w216 = np.asarray(w2).astype(np.float16)
    w316 = np.asarray(w3).astype(np.float16)

    in_maps = []
    for e in range(N_CORES):
        xe = x16[offs[e]:offs[e + 1]]
        if xe.shape[0] < pad:
            xe = np.concatenate(
                [xe, np.zeros((pad - xe.shape[0], D), dtype=np.float16)], axis=0
            )
        in_maps.append({"x16": xe, "w1": w116[e], "w2": w216[e], "w3": w316[e]})
    return nc, in_maps, sizes


def kernel(x, w1, w2, w3, m_sizes):
    from concourse.bass_utils import run_bass_kernel_spmd

    nc, in_maps, sizes = _prep(x, w1, w2, w3, m_sizes)
    r = run_bass_kernel_spmd(nc, in_maps, core_ids=list(range(N_CORES)))
    out = np.concatenate(
        [r.results[e]["out"][: sizes[e]] for e in range(N_CORES)], axis=0
    )
    return out.astype(np.float32)
